# revision 1
# baseline (speedup 1.0000x reference)
"""Trainium2 Bass kernel for AttnDecoderRNN single step (batch=1).

8-way tensor parallel:
  - attention logits row-sharded (48 padded slots/core) -> AllGather of exp(logits)
  - softmax normalization replicated (ones-matmul broadcast + reciprocal)
  - context col-sharded via encoder_outputs column slices (local, no comm)
  - combine (2048x4096) col-sharded -> AllReduce of partial pre-activation
  - GRU (2x 6144x2048) row-sharded over output slots -> local gates
  - final gather of h_new slices / attn weights done on host

All GEMVs run in column orientation on the PE: out[M,1] = lhsT[K,M].T @ rhs[K,1]
with host-pretransposed weight slices, so every intermediate vector lives in
SBUF as [128, n_chunks] column layout and no on-device transposes are needed.
"""

import numpy as np

import concourse.bacc as bacc
import concourse.mybir as mybir
import concourse.tile as tile
from concourse import bass_utils

F32 = mybir.dt.float32
NCORES = 8
H = 2048          # hidden size
L = 350           # max_length
S = 48            # padded attention slots per core (8*48 = 384)
LP = NCORES * S   # padded max_length
HC = H // 128     # 16 column chunks of h / g
ZC = 2 * HC       # 32 column chunks of [x; h]
GS = 3 * (H // NCORES)   # 768 GRU rows per core (r,z,n x 256)
GC = GS // 128    # 6 column chunks of the local gate slices
CS = H // NCORES  # 256: per-core slice of x / ctx / h_new

NEG = -1.0e30

_CACHE = {}


def _build():
    nc = bacc.Bacc(
        "TRN2",
        target_bir_lowering=False,
        debug=False,
        enable_asserts=True,
        num_devices=NCORES,
    )
    rg = [list(range(NCORES))]

    # ---- external inputs (per-core data prepared on host) ----
    d_attn_wt = nc.dram_tensor("attn_wt", [128, ZC * S], F32, kind="ExternalInput")
    d_attn_b = nc.dram_tensor("attn_b", [S, 1], F32, kind="ExternalInput")
    d_z_cols = nc.dram_tensor("z_cols", [128, ZC], F32, kind="ExternalInput")
    d_enc = nc.dram_tensor("enc_cs", [LP // NCORES * NCORES, CS], F32, kind="ExternalInput")  # (384, 256)
    d_comb_wt = nc.dram_tensor("comb_wt", [512, H], F32, kind="ExternalInput")
    d_combx = nc.dram_tensor("combx", [128, 2], F32, kind="ExternalInput")
    d_comb_b = nc.dram_tensor("comb_b", [128, HC], F32, kind="ExternalInput")
    d_wih = nc.dram_tensor("wih_t", [H, GS], F32, kind="ExternalInput")
    d_whh = nc.dram_tensor("whh_t", [H, GS], F32, kind="ExternalInput")
    d_h_cols = nc.dram_tensor("h_cols", [128, HC], F32, kind="ExternalInput")
    d_hsl = nc.dram_tensor("hsl", [128, 2], F32, kind="ExternalInput")
    d_bih = nc.dram_tensor("bih", [128, GC], F32, kind="ExternalInput")
    d_bhh = nc.dram_tensor("bhh", [128, GC], F32, kind="ExternalInput")

    # ---- external outputs ----
    d_h_part = nc.dram_tensor("h_part", [128, 2], F32, kind="ExternalOutput")
    d_aw_part = nc.dram_tensor("aw_part", [S, 1], F32, kind="ExternalOutput")

    ACT = mybir.ActivationFunctionType

    with tile.TileContext(nc) as tc:
        with (
            tc.tile_pool(name="wts", bufs=1) as wp,
            tc.tile_pool(name="work", bufs=1) as wk,
            tc.tile_pool(name="psum", bufs=1, space="PSUM") as ps,
            tc.tile_pool(name="dram", bufs=1, space="DRAM") as dram,
        ):
            # ---------- weight / input DMAs (issue order sets priority) ----------
            # attention weights first (critical-path head), split for queue parallelism
            attn_w = wp.tile([128, ZC * S], F32)
            for i in range(8):
                w = ZC * S // 8
                nc.sync.dma_start(attn_w[:, i * w:(i + 1) * w], d_attn_wt[:, i * w:(i + 1) * w])
            z_cols = wp.tile([128, ZC], F32)
            nc.sync.dma_start(z_cols[:], d_z_cols[:])
            attn_b = wp.tile([S, 1], F32)
            nc.sync.dma_start(attn_b[:], d_attn_b[:])
            enc_sb = []
            for k in range(3):
                t = wp.tile([128, CS], F32, name=f"enc_{k}")
                nc.sync.dma_start(t[:], d_enc[k * 128:(k + 1) * 128, :])
                enc_sb.append(t)
            combx = wp.tile([128, 2], F32)
            nc.sync.dma_start(combx[:], d_combx[:])
            # combine weights: 8 DMAs of 512KB
            comb_sb = []
            for k in range(4):
                t = wp.tile([128, H], F32, name=f"comb_{k}")
                nc.sync.dma_start(t[:, :H // 2], d_comb_wt[k * 128:(k + 1) * 128, :H // 2])
                nc.sync.dma_start(t[:, H // 2:], d_comb_wt[k * 128:(k + 1) * 128, H // 2:])
                comb_sb.append(t)
            h_cols = wp.tile([128, HC], F32)
            nc.sync.dma_start(h_cols[:], d_h_cols[:])
            hsl = wp.tile([128, 2], F32)
            nc.sync.dma_start(hsl[:], d_hsl[:])
            whh_sb = []
            for k in range(HC):
                t = wp.tile([128, GS], F32, name=f"whh_{k}")
                nc.sync.dma_start(t[:], d_whh[k * 128:(k + 1) * 128, :])
                whh_sb.append(t)
            wih_sb = []
            for k in range(HC):
                t = wp.tile([128, GS], F32, name=f"wih_{k}")
                nc.sync.dma_start(t[:], d_wih[k * 128:(k + 1) * 128, :])
                wih_sb.append(t)
            comb_b = wp.tile([128, HC], F32)
            nc.sync.dma_start(comb_b[:], d_comb_b[:])
            bih = wp.tile([128, GC], F32)
            nc.sync.dma_start(bih[:], d_bih[:])
            bhh = wp.tile([128, GC], F32)
            nc.sync.dma_start(bhh[:], d_bhh[:])
            ones = wp.tile([128, 128], F32)
            nc.vector.memset(ones[:], 1.0)

            # ---------- attention logits: [S,1] = attn_W_slice @ [x;h] ----------
            lg_ps = ps.tile([S, 1], F32)
            for k in range(ZC):
                nc.tensor.matmul(
                    lg_ps[:], attn_w[:, k * S:(k + 1) * S], z_cols[:, k:k + 1],
                    start=(k == 0), stop=(k == ZC - 1),
                )
            exp_sb = wk.tile([S, 1], F32)
            nc.scalar.activation(exp_sb[:], lg_ps[:], ACT.Exp, bias=attn_b[:])

            # ---------- AllGather exp(logits) -> all 384 padded slots ----------
            cc1_in = dram.tile([S, 1], F32)
            cc1_out = dram.tile([LP, 1], F32, addr_space="Shared")
            nc.sync.dma_start(cc1_in[:], exp_sb[:])
            nc.gpsimd.collective_compute(
                "AllGather", mybir.AluOpType.bypass, replica_groups=rg,
                ins=[cc1_in[:]], outs=[cc1_out[:]],
            )
            expg = wk.tile([128, 3], F32)
            nc.sync.dma_start(expg[:], cc1_out[:, 0].rearrange("(k p) -> p k", p=128))

            # softmax denominator broadcast to all partitions + reciprocal
            sums_ps = ps.tile([128, 3], F32)
            nc.tensor.matmul(sums_ps[:], ones[:], expg[:], start=True, stop=True)
            tot = wk.tile([128, 1], F32)
            nc.vector.reduce_sum(tot[:], sums_ps[:], axis=mybir.AxisListType.X)
            rcp = wk.tile([128, 1], F32)
            nc.vector.reciprocal(rcp[:], tot[:])
            aw = wk.tile([128, 3], F32)
            nc.vector.tensor_scalar_mul(aw[:], expg[:], rcp[:])
            awp = wk.tile([S, 1], F32)
            nc.vector.tensor_scalar_mul(awp[:], exp_sb[:], rcp[:S, :])
            nc.sync.dma_start(d_aw_part[:], awp[:])

            # ---------- context slice: ctx[256c:256c+256] as [128,2] cols ----------
            ctx_ps = ps.tile([128, 2], F32)
            for m in range(2):
                for k in range(3):
                    nc.tensor.matmul(
                        ctx_ps[:, m:m + 1], enc_sb[k][:, m * 128:(m + 1) * 128],
                        aw[:, k:k + 1], start=(k == 0), stop=(k == 2),
                    )
            ctx = wk.tile([128, 2], F32)
            nc.vector.tensor_copy(ctx[:], ctx_ps[:])

            # ---------- combine partial: g_pre partial [128,16] ----------
            g_ps = ps.tile([128, HC], F32)
            for m in range(HC):
                for k in range(4):
                    rhs = combx[:, k:k + 1] if k < 2 else ctx[:, k - 2:k - 1]
                    nc.tensor.matmul(
                        g_ps[:, m:m + 1], comb_sb[k][:, m * 128:(m + 1) * 128],
                        rhs, start=(k == 0), stop=(k == 3),
                    )
            g_pre = wk.tile([128, HC], F32)
            nc.vector.tensor_copy(g_pre[:], g_ps[:])

            # ---------- gh = w_hh_slice @ h (independent of collectives) ----------
            gh_ps = ps.tile([128, GC], F32)
            for m in range(GC):
                for k in range(HC):
                    nc.tensor.matmul(
                        gh_ps[:, m:m + 1], whh_sb[k][:, m * 128:(m + 1) * 128],
                        h_cols[:, k:k + 1], start=(k == 0), stop=(k == HC - 1),
                    )

            # ---------- AllReduce combine pre-activation ----------
            cc2_in = dram.tile([128, HC], F32)
            cc2_out = dram.tile([128, HC], F32, addr_space="Shared")
            nc.sync.dma_start(cc2_in[:], g_pre[:])
            nc.gpsimd.collective_compute(
                "AllReduce", mybir.AluOpType.add, replica_groups=rg,
                ins=[cc2_in[:]], outs=[cc2_out[:]],
            )
            gsum = wk.tile([128, HC], F32)
            nc.sync.dma_start(gsum[:], cc2_out[:])
            gb = wk.tile([128, HC], F32)
            nc.vector.tensor_add(gb[:], gsum[:], comb_b[:])
            g = wk.tile([128, HC], F32)
            nc.scalar.activation(g[:], gb[:], ACT.Relu)

            # ---------- gi = w_ih_slice @ g ----------
            gi_ps = ps.tile([128, GC], F32)
            for m in range(GC):
                for k in range(HC):
                    nc.tensor.matmul(
                        gi_ps[:, m:m + 1], wih_sb[k][:, m * 128:(m + 1) * 128],
                        g[:, k:k + 1], start=(k == 0), stop=(k == HC - 1),
                    )

            # ---------- GRU gates on the local 256-slot slice ----------
            gihb = wk.tile([128, GC], F32)
            nc.vector.tensor_add(gihb[:], gi_ps[:], bih[:])
            ghhb = wk.tile([128, GC], F32)
            nc.vector.tensor_add(ghhb[:], gh_ps[:], bhh[:])
            rzs = wk.tile([128, 4], F32)
            nc.vector.tensor_add(rzs[:], gihb[:, 0:4], ghhb[:, 0:4])
            rz = wk.tile([128, 4], F32)
            nc.scalar.activation(rz[:], rzs[:], ACT.Sigmoid)
            t1 = wk.tile([128, 2], F32)
            nc.vector.tensor_mul(t1[:], rz[:, 0:2], ghhb[:, 4:6])
            t2 = wk.tile([128, 2], F32)
            nc.vector.tensor_add(t2[:], t1[:], gihb[:, 4:6])
            nt = wk.tile([128, 2], F32)
            nc.scalar.activation(nt[:], t2[:], ACT.Tanh)
            hmn = wk.tile([128, 2], F32)
            nc.vector.tensor_sub(hmn[:], hsl[:], nt[:])
            zt = wk.tile([128, 2], F32)
            nc.vector.tensor_mul(zt[:], rz[:, 2:4], hmn[:])
            hnew = wk.tile([128, 2], F32)
            nc.vector.tensor_add(hnew[:], nt[:], zt[:])
            nc.sync.dma_start(d_h_part[:], hnew[:])

    nc.compile()
    return nc


def _prep(inputs):
    """Build per-core input maps from the full problem inputs."""
    f = lambda a: np.ascontiguousarray(np.asarray(a, dtype=np.float32))
    x = f(inputs["input"]).reshape(H)
    h = f(inputs["hidden"]).reshape(H)
    enc = f(inputs["encoder_outputs"])
    attn_W = f(inputs["attn_W"])
    attn_b = f(inputs["attn_b"])
    comb_W = f(inputs["comb_W"])
    comb_b = f(inputs["comb_b"])
    w_ih = f(inputs["w_ih"])
    w_hh = f(inputs["w_hh"])
    b_ih = f(inputs["b_ih"])
    b_hh = f(inputs["b_hh"])

    z = np.concatenate([x, h])
    z_cols = np.ascontiguousarray(z.reshape(ZC, 128).T)
    h_cols = np.ascontiguousarray(h.reshape(HC, 128).T)

    Wp = np.zeros((LP, 2 * H), np.float32)
    Wp[:L] = attn_W
    bp = np.full((LP,), NEG, np.float32)
    bp[:L] = attn_b
    encp = np.zeros((LP, H), np.float32)
    encp[:L] = enc
    comb_WT = comb_W.T  # (4096, 2048)
    comb_bt = np.ascontiguousarray(comb_b.reshape(HC, 128).T)

    in_maps = []
    for c in range(NCORES):
        WcT = Wp[c * S:(c + 1) * S].T  # (4096, S)
        attn_wt = np.ascontiguousarray(
            WcT.reshape(ZC, 128, S).transpose(1, 0, 2).reshape(128, ZC * S))
        sel = np.concatenate([np.arange(c * CS, (c + 1) * CS) + g * H for g in range(3)])
        in_maps.append({
            "attn_wt": attn_wt,
            "attn_b": np.ascontiguousarray(bp[c * S:(c + 1) * S].reshape(S, 1)),
            "z_cols": z_cols,
            "enc_cs": np.ascontiguousarray(encp[:, c * CS:(c + 1) * CS]),
            "comb_wt": np.ascontiguousarray(np.concatenate(
                [comb_WT[c * CS:(c + 1) * CS], comb_WT[H + c * CS:H + (c + 1) * CS]], axis=0)),
            "combx": np.ascontiguousarray(x[c * CS:(c + 1) * CS].reshape(2, 128).T),
            "comb_b": comb_bt,
            "wih_t": np.ascontiguousarray(w_ih[sel].T),
            "whh_t": np.ascontiguousarray(w_hh[sel].T),
            "h_cols": h_cols,
            "hsl": np.ascontiguousarray(h[c * CS:(c + 1) * CS].reshape(2, 128).T),
            "bih": np.ascontiguousarray(b_ih[sel].reshape(GC, 128).T),
            "bhh": np.ascontiguousarray(b_hh[sel].reshape(GC, 128).T),
        })
    return in_maps


def kernel(**inputs):
    if "nc" not in _CACHE:
        _CACHE["nc"] = _build()
    nc = _CACHE["nc"]
    in_maps = _prep(inputs)
    res = bass_utils.run_bass_kernel_spmd(
        nc, in_maps, core_ids=list(range(NCORES)), **_CACHE.get("run_kwargs", {}))
    _CACHE["last_result"] = res

    h_full = np.concatenate(
        [np.ascontiguousarray(res.results[c]["h_part"]).T.reshape(CS) for c in range(NCORES)])
    aw_full = np.concatenate(
        [np.ascontiguousarray(res.results[c]["aw_part"]).reshape(S) for c in range(NCORES)])[:L]
    out = h_full.reshape(1, 1, H).astype(np.float32)
    return (out, out.copy(), aw_full.reshape(1, L).astype(np.float32))


# revision 5
# speedup vs baseline: 1.3139x; 1.3139x over previous
"""Trainium2 Bass kernel for AttnDecoderRNN single step (batch=1).

8-way tensor parallel:
  - attention logits row-sharded (48 padded slots/core) -> AllGather of exp(logits)
  - softmax normalization replicated (ones-matmul broadcast + reciprocal)
  - context col-sharded via encoder_outputs column slices (local, no comm)
  - combine (2048x4096) col-sharded -> AllReduce of partial pre-activation
  - GRU (2x 6144x2048) row-sharded over output slots -> local gates
  - final gather of h_new slices / attn weights done on host

GEMV orientation: the big weight matrices stream through the PE as the MOVING
operand (rhs, N<=512 fp32) with the input vector chunk as the stationary lhsT
([128,1], trivial LDWEIGHTS).  Only the small context matmul keeps the
weights-stationary orientation to produce column-layout output directly.
The g vector crossing the AllReduce uses an interleaved chunk layout
(chunk k = indices {16p+k}) so both DRAM<->SBUF DMAs stay contiguous; the
GRU weight rows are permuted on the host to match.
"""

import numpy as np

import concourse.bacc as bacc
import concourse.bass as bass
import concourse.mybir as mybir
import concourse.tile as tile
from concourse import bass_utils

F32 = mybir.dt.float32
NCORES = 8
H = 2048          # hidden size
L = 350           # max_length
S = 48            # padded attention slots per core (8*48 = 384)
LP = NCORES * S   # padded max_length
HC = H // 128     # 16 column chunks of h / g
ZC = 2 * HC       # 32 column chunks of [x; h]
GS = 3 * (H // NCORES)   # 768 GRU rows per core (r,z,n x 256)
CS = H // NCORES  # 256: per-core slice of x / ctx / h_new

NEG = -1.0e30

_CACHE = {}


def _build():
    nc = bacc.Bacc(
        "TRN2",
        target_bir_lowering=False,
        debug=False,
        enable_asserts=True,
        num_devices=NCORES,
    )
    rg = [list(range(NCORES))]

    # ---- external inputs (per-core data prepared on host) ----
    d_attn_wt = nc.dram_tensor("attn_wt", [128, ZC * S], F32, kind="ExternalInput")
    d_attn_b = nc.dram_tensor("attn_b", [1, S], F32, kind="ExternalInput")
    d_z_cols = nc.dram_tensor("z_cols", [128, ZC], F32, kind="ExternalInput")
    d_enc = nc.dram_tensor("enc_cs", [LP, CS], F32, kind="ExternalInput")  # (384, 256)
    d_comb_wt = nc.dram_tensor("comb_wt", [512, H], F32, kind="ExternalInput")
    d_combx = nc.dram_tensor("combx", [128, 2], F32, kind="ExternalInput")
    d_comb_b = nc.dram_tensor("comb_b", [128, HC], F32, kind="ExternalInput")
    d_wih = nc.dram_tensor("wih_t", [H, GS], F32, kind="ExternalInput")
    d_whh = nc.dram_tensor("whh_t", [H, GS], F32, kind="ExternalInput")
    d_h_cols = nc.dram_tensor("h_cols", [128, HC], F32, kind="ExternalInput")
    d_hsl = nc.dram_tensor("hsl", [1, CS], F32, kind="ExternalInput")
    d_bih = nc.dram_tensor("bih", [1, GS], F32, kind="ExternalInput")
    d_bhh = nc.dram_tensor("bhh", [1, GS], F32, kind="ExternalInput")

    # ---- external outputs ----
    d_h_part = nc.dram_tensor("h_part", [1, CS], F32, kind="ExternalOutput")
    d_aw_part = nc.dram_tensor("aw_part", [1, S], F32, kind="ExternalOutput")

    ACT = mybir.ActivationFunctionType

    with tile.TileContext(nc) as tc:
        with (
            tc.tile_pool(name="wts", bufs=1) as wp,
            tc.tile_pool(name="work", bufs=1) as wk,
            tc.tile_pool(name="psum", bufs=1, space="PSUM") as ps,
            tc.tile_pool(name="dram", bufs=1, space="DRAM") as dram,
        ):
            # ---------- weight / input DMAs (issue order sets priority) ----------
            attn_w = wp.tile([128, ZC * S], F32)
            for i in range(8):
                w = ZC * S // 8
                nc.sync.dma_start(attn_w[:, i * w:(i + 1) * w], d_attn_wt[:, i * w:(i + 1) * w])
            z_cols = wp.tile([128, ZC], F32)
            nc.sync.dma_start(z_cols[:], d_z_cols[:])
            attn_b = wp.tile([1, S], F32)
            nc.sync.dma_start(attn_b[:], d_attn_b[:])
            enc_sb = []
            for k in range(3):
                t = wp.tile([128, CS], F32, name=f"enc_{k}")
                nc.sync.dma_start(t[:], d_enc[k * 128:(k + 1) * 128, :])
                enc_sb.append(t)
            combx = wp.tile([128, 2], F32)
            nc.sync.dma_start(combx[:], d_combx[:])
            # combine weights: 8 DMAs of 512KB
            comb_sb = []
            for k in range(4):
                t = wp.tile([128, H], F32, name=f"comb_{k}")
                nc.sync.dma_start(t[:, :H // 2], d_comb_wt[k * 128:(k + 1) * 128, :H // 2])
                nc.sync.dma_start(t[:, H // 2:], d_comb_wt[k * 128:(k + 1) * 128, H // 2:])
                comb_sb.append(t)
            h_cols = wp.tile([128, HC], F32)
            nc.sync.dma_start(h_cols[:], d_h_cols[:])
            hsl = wp.tile([1, CS], F32)
            nc.sync.dma_start(hsl[:], d_hsl[:])
            whh_sb = []
            for k in range(HC):
                t = wp.tile([128, GS], F32, name=f"whh_{k}")
                nc.sync.dma_start(t[:], d_whh[k * 128:(k + 1) * 128, :])
                whh_sb.append(t)
            wih_sb = []
            for k in range(HC):
                t = wp.tile([128, GS], F32, name=f"wih_{k}")
                nc.sync.dma_start(t[:], d_wih[k * 128:(k + 1) * 128, :])
                wih_sb.append(t)
            comb_b = wp.tile([128, HC], F32)
            nc.sync.dma_start(comb_b[:], d_comb_b[:])
            bih = wp.tile([1, GS], F32)
            nc.sync.dma_start(bih[:], d_bih[:])
            bhh = wp.tile([1, GS], F32)
            nc.sync.dma_start(bhh[:], d_bhh[:])
            ones = wp.tile([128, 128], F32)
            nc.vector.memset(ones[:], 1.0)

            # ---------- attention logits row: [1,S] = (attn_W_slice @ [x;h]).T ----------
            lg_ps = ps.tile([1, S], F32, tag="sp", bufs=1)
            for k in range(ZC):
                nc.tensor.matmul(
                    lg_ps[:], z_cols[:, k:k + 1], attn_w[:, k * S:(k + 1) * S],
                    start=(k == 0), stop=(k == ZC - 1),
                )
            lgb = wk.tile([1, S], F32)
            nc.vector.tensor_add(lgb[:], lg_ps[:], attn_b[:])
            exp_sb = wk.tile([1, S], F32)
            nc.scalar.activation(exp_sb[:], lgb[:], ACT.Exp)

            # ---------- AllGather exp(logits) -> all 384 padded slots ----------
            cc1_in = dram.tile([1, S], F32)
            cc1_out = dram.tile([1, LP], F32, addr_space="Shared")
            nc.sync.dma_start(cc1_in[:], exp_sb[:])
            nc.gpsimd.collective_compute(
                "AllGather", mybir.AluOpType.bypass, replica_groups=rg,
                ins=[cc1_in[:]], outs=[cc1_out[:]],
            )
            expg = wk.tile([128, 3], F32)
            nc.sync.dma_start(expg[:], cc1_out[0, :].rearrange("(k p) -> p k", p=128))

            # softmax denominator broadcast to all partitions + reciprocal
            sums_ps = ps.tile([128, 3], F32, tag="sp", bufs=1)
            nc.tensor.matmul(sums_ps[:], ones[:], expg[:], start=True, stop=True)
            tot = wk.tile([128, 1], F32)
            nc.vector.reduce_sum(tot[:], sums_ps[:], axis=mybir.AxisListType.X)
            rcp = wk.tile([128, 1], F32)
            nc.vector.reciprocal(rcp[:], tot[:])
            aw = wk.tile([128, 3], F32)
            nc.vector.tensor_scalar_mul(aw[:], expg[:], rcp[:])
            awp = wk.tile([1, S], F32)
            nc.vector.tensor_scalar_mul(awp[:], exp_sb[:], rcp[0:1, :])
            nc.sync.dma_start(d_aw_part[:], awp[:])

            # ---------- context slice: ctx[256c:256c+256] as [128,2] cols ----------
            # weights-stationary so the output lands in column layout directly
            ctx_ps = ps.tile([128, 2], F32, tag="sp", bufs=1)
            for m in range(2):
                for k in range(3):
                    nc.tensor.matmul(
                        ctx_ps[:, m:m + 1], enc_sb[k][:, m * 128:(m + 1) * 128],
                        aw[:, k:k + 1], start=(k == 0), stop=(k == 2),
                    )
            ctx = wk.tile([128, 2], F32)
            nc.vector.tensor_copy(ctx[:], ctx_ps[:])

            # ---------- combine partial: g_pre row [1,2048] in 4 psum banks ----------
            g_ps = [ps.tile([1, 512], F32, name=f"g_ps{j}", tag="gp", bufs=2)
                    for j in range(4)]
            for j in range(4):
                for k in range(4):
                    lhs = combx[:, k:k + 1] if k < 2 else ctx[:, k - 2:k - 1]
                    nc.tensor.matmul(
                        g_ps[j][:], lhs, comb_sb[k][:, j * 512:(j + 1) * 512],
                        start=(k == 0), stop=(k == 3),
                    )

            # ---------- gh row = (w_hh_slice @ h).T  (independent of collectives) ----
            gh_ps_a = ps.tile([1, 512], F32, tag="gha")
            gh_ps_b = ps.tile([1, GS - 512], F32, tag="ghb")
            for (t, n0, nw) in ((gh_ps_a, 0, 512), (gh_ps_b, 512, GS - 512)):
                for k in range(HC):
                    nc.tensor.matmul(
                        t[:], h_cols[:, k:k + 1], whh_sb[k][:, n0:n0 + nw],
                        start=(k == 0), stop=(k == HC - 1),
                    )

            # ---------- AllReduce combine pre-activation ----------
            cc2_in = dram.tile([1, H], F32)
            cc2_out = dram.tile([1, H], F32, addr_space="Shared")
            g_pre = wk.tile([1, H], F32)
            for j in range(4):
                if j % 2 == 0:
                    nc.vector.tensor_copy(g_pre[:, j * 512:(j + 1) * 512], g_ps[j][:])
                else:
                    nc.scalar.copy(g_pre[:, j * 512:(j + 1) * 512], g_ps[j][:])
            nc.sync.dma_start(cc2_in[:], g_pre[:])
            nc.gpsimd.collective_compute(
                "AllReduce", mybir.AluOpType.add, replica_groups=rg,
                ins=[cc2_in[:]], outs=[cc2_out[:]],
            )
            # load summed g_pre as [128,16]: chunk k holds indices {16p+k}
            gsum = wk.tile([128, HC], F32)
            nc.sync.dma_start(gsum[:], cc2_out[0, :].rearrange("(p k) -> p k", k=HC))
            gb = wk.tile([128, HC], F32)
            nc.vector.tensor_add(gb[:], gsum[:], comb_b[:])
            g = wk.tile([128, HC], F32)
            nc.scalar.activation(g[:], gb[:], ACT.Relu)

            # ---------- gi row = (w_ih_slice @ g).T ----------
            gi_ps_a = ps.tile([1, 512], F32, tag="gia")
            gi_ps_b = ps.tile([1, GS - 512], F32, tag="gib")
            for (t, n0, nw) in ((gi_ps_a, 0, 512), (gi_ps_b, 512, GS - 512)):
                for k in range(HC):
                    nc.tensor.matmul(
                        t[:], g[:, k:k + 1], wih_sb[k][:, n0:n0 + nw],
                        start=(k == 0), stop=(k == HC - 1),
                    )

            # ---------- GRU gates on the local 256-slot slice (row layout) -------
            gihb = wk.tile([1, GS], F32)
            nc.vector.tensor_add(gihb[:, 0:512], gi_ps_a[:], bih[:, 0:512])
            nc.vector.tensor_add(gihb[:, 512:GS], gi_ps_b[:], bih[:, 512:GS])
            ghhb = wk.tile([1, GS], F32)
            nc.vector.tensor_add(ghhb[:, 0:512], gh_ps_a[:], bhh[:, 0:512])
            nc.vector.tensor_add(ghhb[:, 512:GS], gh_ps_b[:], bhh[:, 512:GS])
            rzs = wk.tile([1, 512], F32)
            nc.vector.tensor_add(rzs[:], gihb[:, 0:512], ghhb[:, 0:512])
            rz = wk.tile([1, 512], F32)
            nc.scalar.activation(rz[:], rzs[:], ACT.Sigmoid)
            t1 = wk.tile([1, CS], F32)
            nc.vector.tensor_mul(t1[:], rz[:, 0:CS], ghhb[:, 512:GS])
            t2 = wk.tile([1, CS], F32)
            nc.vector.tensor_add(t2[:], t1[:], gihb[:, 512:GS])
            nt = wk.tile([1, CS], F32)
            nc.scalar.activation(nt[:], t2[:], ACT.Tanh)
            hmn = wk.tile([1, CS], F32)
            nc.vector.tensor_sub(hmn[:], hsl[:], nt[:])
            zt = wk.tile([1, CS], F32)
            nc.vector.tensor_mul(zt[:], rz[:, CS:512], hmn[:])
            hnew = wk.tile([1, CS], F32)
            nc.vector.tensor_add(hnew[:], nt[:], zt[:])
            nc.sync.dma_start(d_h_part[:], hnew[:])

    nc.compile()
    return nc


def _prep(inputs):
    """Build per-core input maps from the full problem inputs."""
    f = lambda a: np.ascontiguousarray(np.asarray(a, dtype=np.float32))
    x = f(inputs["input"]).reshape(H)
    h = f(inputs["hidden"]).reshape(H)
    enc = f(inputs["encoder_outputs"])
    attn_W = f(inputs["attn_W"])
    attn_b = f(inputs["attn_b"])
    comb_W = f(inputs["comb_W"])
    comb_b = f(inputs["comb_b"])
    w_ih = f(inputs["w_ih"])
    w_hh = f(inputs["w_hh"])
    b_ih = f(inputs["b_ih"])
    b_hh = f(inputs["b_hh"])

    z = np.concatenate([x, h])
    z_cols = np.ascontiguousarray(z.reshape(ZC, 128).T)
    h_cols = np.ascontiguousarray(h.reshape(128, HC))       # interleaved chunks
    comb_bt = np.ascontiguousarray(comb_b.reshape(128, HC))  # interleaved chunks

    Wp = np.zeros((LP, 2 * H), np.float32)
    Wp[:L] = attn_W
    bp = np.full((LP,), NEG, np.float32)
    bp[:L] = attn_b
    encp = np.zeros((LP, H), np.float32)
    encp[:L] = enc
    comb_WT = comb_W.T  # (4096, 2048)
    # row permutation so k-chunk k of the GRU contraction = g indices {16p+k}
    perm = np.add.outer(np.arange(HC), HC * np.arange(128)).reshape(-1)

    in_maps = []
    for c in range(NCORES):
        WcT = Wp[c * S:(c + 1) * S].T  # (4096, S)
        attn_wt = np.ascontiguousarray(
            WcT.reshape(ZC, 128, S).transpose(1, 0, 2).reshape(128, ZC * S))
        sel = np.concatenate([np.arange(c * CS, (c + 1) * CS) + g * H for g in range(3)])
        in_maps.append({
            "attn_wt": attn_wt,
            "attn_b": np.ascontiguousarray(bp[c * S:(c + 1) * S].reshape(1, S)),
            "z_cols": z_cols,
            "enc_cs": np.ascontiguousarray(encp[:, c * CS:(c + 1) * CS]),
            "comb_wt": np.ascontiguousarray(np.concatenate(
                [comb_WT[c * CS:(c + 1) * CS], comb_WT[H + c * CS:H + (c + 1) * CS]], axis=0)),
            "combx": np.ascontiguousarray(x[c * CS:(c + 1) * CS].reshape(2, 128).T),
            "comb_b": comb_bt,
            "wih_t": np.ascontiguousarray(w_ih[sel].T[perm]),
            "whh_t": np.ascontiguousarray(w_hh[sel].T[perm]),
            "h_cols": h_cols,
            "hsl": np.ascontiguousarray(h[c * CS:(c + 1) * CS].reshape(1, CS)),
            "bih": np.ascontiguousarray(b_ih[sel].reshape(1, GS)),
            "bhh": np.ascontiguousarray(b_hh[sel].reshape(1, GS)),
        })
    return in_maps


def kernel(**inputs):
    if "nc" not in _CACHE:
        _CACHE["nc"] = _build()
    nc = _CACHE["nc"]
    in_maps = _prep(inputs)
    res = bass_utils.run_bass_kernel_spmd(
        nc, in_maps, core_ids=list(range(NCORES)), **_CACHE.get("run_kwargs", {}))
    _CACHE["last_result"] = res

    h_full = np.concatenate(
        [np.asarray(res.results[c]["h_part"]).reshape(CS) for c in range(NCORES)])
    aw_full = np.concatenate(
        [np.asarray(res.results[c]["aw_part"]).reshape(S) for c in range(NCORES)])[:L]
    out = h_full.reshape(1, 1, H).astype(np.float32)
    return (out, out.copy(), aw_full.reshape(1, L).astype(np.float32))


# revision 6
# speedup vs baseline: 2.0633x; 1.5703x over previous
"""Trainium2 Bass kernel for AttnDecoderRNN single step (batch=1).

8-way tensor parallel, ONE on-device collective:
  - attention fully replicated per core (attn_W is small): logits + softmax local
  - context col-sharded via encoder_outputs column slices (local, no comm)
  - combine (2048x4096) col-sharded -> AllReduce of partial pre-activation (fp32)
  - GRU (2x 6144x2048) row-sharded over output slots -> local gates
  - final gather of h_new slices done on host; attn weights from core 0

Matmul operands are fp16 (single-pass on the PE, half the HBM traffic);
accumulation is fp32 in PSUM and all vector/scalar math including the
AllReduce stays fp32.  The big weight matrices stream through the PE as the
MOVING operand (rhs, N<=512) with the input-vector chunk as the stationary
lhsT ([128,1], trivial LDWEIGHTS).  The g vector crossing the AllReduce uses
an interleaved chunk layout (chunk k = indices {16p+k}) so both DRAM<->SBUF
DMAs stay contiguous; GRU weight rows are permuted on the host to match.
"""

import numpy as np

import concourse.bacc as bacc
import concourse.bass as bass
import concourse.mybir as mybir
import concourse.tile as tile
from concourse import bass_utils

F32 = mybir.dt.float32
F16 = mybir.dt.float16
NP16 = np.float16
NCORES = 8
H = 2048          # hidden size
L = 350           # max_length
LP = 384          # padded max_length (3 x 128)
HC = H // 128     # 16 column chunks of h / g
ZC = 2 * HC       # 32 column chunks of [x; h]
GS = 3 * (H // NCORES)   # 768 GRU rows per core (r,z,n x 256)
CS = H // NCORES  # 256: per-core slice of x / ctx / h_new

NEG = -1.0e30

_CACHE = {}


def _build():
    nc = bacc.Bacc(
        "TRN2",
        target_bir_lowering=False,
        debug=False,
        enable_asserts=True,
        num_devices=NCORES,
    )
    rg = [list(range(NCORES))]

    # ---- external inputs (per-core data prepared on host) ----
    d_attn_wt = nc.dram_tensor("attn_wt", [128, ZC * LP], F16, kind="ExternalInput")
    d_attn_b = nc.dram_tensor("attn_b", [1, LP], F32, kind="ExternalInput")
    d_z_cols = nc.dram_tensor("z_cols", [128, ZC], F16, kind="ExternalInput")
    d_enc = nc.dram_tensor("enc_cs", [LP, CS], F16, kind="ExternalInput")
    d_comb_wt = nc.dram_tensor("comb_wt", [512, H], F16, kind="ExternalInput")
    d_combx = nc.dram_tensor("combx", [128, 2], F16, kind="ExternalInput")
    d_comb_b = nc.dram_tensor("comb_b", [128, HC], F32, kind="ExternalInput")
    d_wih = nc.dram_tensor("wih_t", [H, GS], F16, kind="ExternalInput")
    d_whh = nc.dram_tensor("whh_t", [H, GS], F16, kind="ExternalInput")
    d_h_cols = nc.dram_tensor("h_cols", [128, HC], F16, kind="ExternalInput")
    d_hsl = nc.dram_tensor("hsl", [1, CS], F32, kind="ExternalInput")
    d_bih = nc.dram_tensor("bih", [1, GS], F32, kind="ExternalInput")
    d_bhh = nc.dram_tensor("bhh", [1, GS], F32, kind="ExternalInput")

    # ---- external outputs ----
    d_h_part = nc.dram_tensor("h_part", [1, CS], F32, kind="ExternalOutput")
    d_aw = nc.dram_tensor("aw_full", [1, LP], F32, kind="ExternalOutput")

    ACT = mybir.ActivationFunctionType

    with tile.TileContext(nc) as tc:
        with (
            tc.tile_pool(name="wts", bufs=1) as wp,
            tc.tile_pool(name="work", bufs=1) as wk,
            tc.tile_pool(name="psum", bufs=1, space="PSUM") as ps,
            tc.tile_pool(name="dram", bufs=1, space="DRAM") as dram,
        ):
            # ---------- weight / input DMAs (issue order sets priority) ----------
            z_cols = wp.tile([128, ZC], F16)
            nc.sync.dma_start(z_cols[:], d_z_cols[:])
            attn_b = wp.tile([1, LP], F32)
            nc.sync.dma_start(attn_b[:], d_attn_b[:])
            attn_w = wp.tile([128, ZC * LP], F16)
            for i in range(8):
                w = ZC * LP // 8
                nc.sync.dma_start(attn_w[:, i * w:(i + 1) * w], d_attn_wt[:, i * w:(i + 1) * w])
            enc_sb = []
            for k in range(3):
                t = wp.tile([128, CS], F16, name=f"enc_{k}")
                nc.sync.dma_start(t[:], d_enc[k * 128:(k + 1) * 128, :])
                enc_sb.append(t)
            combx = wp.tile([128, 2], F16)
            nc.sync.dma_start(combx[:], d_combx[:])
            comb_sb = []
            for k in range(4):
                t = wp.tile([128, H], F16, name=f"comb_{k}")
                nc.sync.dma_start(t[:, :H // 2], d_comb_wt[k * 128:(k + 1) * 128, :H // 2])
                nc.sync.dma_start(t[:, H // 2:], d_comb_wt[k * 128:(k + 1) * 128, H // 2:])
                comb_sb.append(t)
            h_cols = wp.tile([128, HC], F16)
            nc.sync.dma_start(h_cols[:], d_h_cols[:])
            hsl = wp.tile([1, CS], F32)
            nc.sync.dma_start(hsl[:], d_hsl[:])
            whh_sb = []
            for k in range(HC):
                t = wp.tile([128, GS], F16, name=f"whh_{k}")
                nc.sync.dma_start(t[:], d_whh[k * 128:(k + 1) * 128, :])
                whh_sb.append(t)
            wih_sb = []
            for k in range(HC):
                t = wp.tile([128, GS], F16, name=f"wih_{k}")
                nc.sync.dma_start(t[:], d_wih[k * 128:(k + 1) * 128, :])
                wih_sb.append(t)
            comb_b = wp.tile([128, HC], F32)
            nc.sync.dma_start(comb_b[:], d_comb_b[:])
            bih = wp.tile([1, GS], F32)
            nc.sync.dma_start(bih[:], d_bih[:])
            bhh = wp.tile([1, GS], F32)
            nc.sync.dma_start(bhh[:], d_bhh[:])
            one1 = wp.tile([1, 1], F16)
            nc.vector.memset(one1[:], 1.0)

            # ---------- attention logits (replicated): [1,LP] ----------
            lg_ps = ps.tile([1, LP], F32, tag="sp", bufs=1)
            for k in range(ZC):
                nc.tensor.matmul(
                    lg_ps[:], z_cols[:, k:k + 1], attn_w[:, k * LP:(k + 1) * LP],
                    start=(k == 0), stop=(k == ZC - 1),
                )
            lgb = wk.tile([1, LP], F32)
            nc.vector.tensor_add(lgb[:], lg_ps[:], attn_b[:])
            exp_row = wk.tile([1, LP], F32)
            nc.scalar.activation(exp_row[:], lgb[:], ACT.Exp)

            # softmax normalization (row layout, replicated)
            tot = wk.tile([1, 1], F32)
            nc.vector.reduce_sum(tot[:], exp_row[:], axis=mybir.AxisListType.X)
            rcp = wk.tile([1, 1], F32)
            nc.vector.reciprocal(rcp[:], tot[:])
            aw_row = wk.tile([1, LP], F32)
            nc.vector.tensor_scalar_mul(aw_row[:], exp_row[:], rcp[:])
            nc.sync.dma_start(d_aw[:], aw_row[:])
            aw16 = wk.tile([1, LP], F16)
            nc.vector.tensor_copy(aw16[:], aw_row[:])

            # transpose aw row -> 3 column chunks via K=1 matmuls
            awc_ps = ps.tile([128, 3], F32, tag="sp", bufs=1)
            for k in range(3):
                nc.tensor.matmul(awc_ps[:, k:k + 1], aw16[0:1, k * 128:(k + 1) * 128],
                                 one1[:], start=True, stop=True)
            aw_cols = wk.tile([128, 3], F16)
            nc.vector.tensor_copy(aw_cols[:], awc_ps[:])

            # ---------- context slice: ctx[256c:256c+256] as [128,2] cols ----------
            ctx_ps = ps.tile([128, 2], F32, tag="sp", bufs=1)
            for m in range(2):
                for k in range(3):
                    nc.tensor.matmul(
                        ctx_ps[:, m:m + 1], enc_sb[k][:, m * 128:(m + 1) * 128],
                        aw_cols[:, k:k + 1], start=(k == 0), stop=(k == 2),
                    )
            ctx = wk.tile([128, 2], F16)
            nc.vector.tensor_copy(ctx[:], ctx_ps[:])

            # ---------- combine partial: g_pre row [1,2048] in 4 psum banks ----------
            g_ps = [ps.tile([1, 512], F32, name=f"g_ps{j}", tag="gp", bufs=2)
                    for j in range(4)]
            for j in range(4):
                for k in range(4):
                    lhs = combx[:, k:k + 1] if k < 2 else ctx[:, k - 2:k - 1]
                    nc.tensor.matmul(
                        g_ps[j][:], lhs, comb_sb[k][:, j * 512:(j + 1) * 512],
                        start=(k == 0), stop=(k == 3),
                    )

            # ---------- gh row = (w_hh_slice @ h).T  (independent of collective) ----
            gh_ps_a = ps.tile([1, 512], F32, tag="gha")
            gh_ps_b = ps.tile([1, GS - 512], F32, tag="ghb")
            for (t, n0, nw) in ((gh_ps_a, 0, 512), (gh_ps_b, 512, GS - 512)):
                for k in range(HC):
                    nc.tensor.matmul(
                        t[:], h_cols[:, k:k + 1], whh_sb[k][:, n0:n0 + nw],
                        start=(k == 0), stop=(k == HC - 1),
                    )

            # ---------- AllReduce combine pre-activation (fp32) ----------
            cc2_in = dram.tile([1, H], F32)
            cc2_out = dram.tile([1, H], F32, addr_space="Shared")
            g_pre = wk.tile([1, H], F32)
            for j in range(4):
                if j % 2 == 0:
                    nc.vector.tensor_copy(g_pre[:, j * 512:(j + 1) * 512], g_ps[j][:])
                else:
                    nc.scalar.copy(g_pre[:, j * 512:(j + 1) * 512], g_ps[j][:])
            nc.sync.dma_start(cc2_in[:], g_pre[:])
            nc.gpsimd.collective_compute(
                "AllReduce", mybir.AluOpType.add, replica_groups=rg,
                ins=[cc2_in[:]], outs=[cc2_out[:]],
            )
            # load summed g_pre as [128,16]: chunk k holds indices {16p+k}
            gsum = wk.tile([128, HC], F32)
            nc.sync.dma_start(gsum[:], cc2_out[0, :].rearrange("(p k) -> p k", k=HC))
            gb = wk.tile([128, HC], F32)
            nc.vector.tensor_add(gb[:], gsum[:], comb_b[:])
            g = wk.tile([128, HC], F16)
            nc.scalar.activation(g[:], gb[:], ACT.Relu)

            # ---------- gi row = (w_ih_slice @ g).T ----------
            gi_ps_a = ps.tile([1, 512], F32, tag="gia")
            gi_ps_b = ps.tile([1, GS - 512], F32, tag="gib")
            for (t, n0, nw) in ((gi_ps_a, 0, 512), (gi_ps_b, 512, GS - 512)):
                for k in range(HC):
                    nc.tensor.matmul(
                        t[:], g[:, k:k + 1], wih_sb[k][:, n0:n0 + nw],
                        start=(k == 0), stop=(k == HC - 1),
                    )

            # ---------- GRU gates on the local 256-slot slice (row layout) -------
            gihb = wk.tile([1, GS], F32)
            nc.vector.tensor_add(gihb[:, 0:512], gi_ps_a[:], bih[:, 0:512])
            nc.vector.tensor_add(gihb[:, 512:GS], gi_ps_b[:], bih[:, 512:GS])
            ghhb = wk.tile([1, GS], F32)
            nc.vector.tensor_add(ghhb[:, 0:512], gh_ps_a[:], bhh[:, 0:512])
            nc.vector.tensor_add(ghhb[:, 512:GS], gh_ps_b[:], bhh[:, 512:GS])
            rzs = wk.tile([1, 512], F32)
            nc.vector.tensor_add(rzs[:], gihb[:, 0:512], ghhb[:, 0:512])
            rz = wk.tile([1, 512], F32)
            nc.scalar.activation(rz[:], rzs[:], ACT.Sigmoid)
            t1 = wk.tile([1, CS], F32)
            nc.vector.tensor_mul(t1[:], rz[:, 0:CS], ghhb[:, 512:GS])
            t2 = wk.tile([1, CS], F32)
            nc.vector.tensor_add(t2[:], t1[:], gihb[:, 512:GS])
            nt = wk.tile([1, CS], F32)
            nc.scalar.activation(nt[:], t2[:], ACT.Tanh)
            hmn = wk.tile([1, CS], F32)
            nc.vector.tensor_sub(hmn[:], hsl[:], nt[:])
            zt = wk.tile([1, CS], F32)
            nc.vector.tensor_mul(zt[:], rz[:, CS:512], hmn[:])
            hnew = wk.tile([1, CS], F32)
            nc.vector.tensor_add(hnew[:], nt[:], zt[:])
            nc.sync.dma_start(d_h_part[:], hnew[:])

    nc.compile()
    return nc


def _prep(inputs):
    """Build per-core input maps from the full problem inputs."""
    f = lambda a: np.ascontiguousarray(np.asarray(a, dtype=np.float32))
    x = f(inputs["input"]).reshape(H)
    h = f(inputs["hidden"]).reshape(H)
    enc = f(inputs["encoder_outputs"])
    attn_W = f(inputs["attn_W"])
    attn_b = f(inputs["attn_b"])
    comb_W = f(inputs["comb_W"])
    comb_b = f(inputs["comb_b"])
    w_ih = f(inputs["w_ih"])
    w_hh = f(inputs["w_hh"])
    b_ih = f(inputs["b_ih"])
    b_hh = f(inputs["b_hh"])

    z = np.concatenate([x, h])
    z_cols = np.ascontiguousarray(z.reshape(ZC, 128).T.astype(NP16))
    h_cols = np.ascontiguousarray(h.astype(NP16).reshape(128, HC))   # interleaved
    comb_bt = np.ascontiguousarray(comb_b.reshape(128, HC))          # interleaved

    Wp = np.zeros((LP, 2 * H), np.float32)
    Wp[:L] = attn_W
    bp = np.full((1, LP), NEG, np.float32)
    bp[0, :L] = attn_b
    encp = np.zeros((LP, H), np.float32)
    encp[:L] = enc
    # replicated attention weights, packed for k-chunked rhs access
    attn_wt = np.ascontiguousarray(
        Wp.T.reshape(ZC, 128, LP).transpose(1, 0, 2).reshape(128, ZC * LP).astype(NP16))
    comb_WT = comb_W.T  # (4096, 2048)
    # row permutation so k-chunk k of the GRU contraction = g indices {16p+k}
    perm = np.add.outer(np.arange(HC), HC * np.arange(128)).reshape(-1)

    in_maps = []
    for c in range(NCORES):
        sel = np.concatenate([np.arange(c * CS, (c + 1) * CS) + g * H for g in range(3)])
        in_maps.append({
            "attn_wt": attn_wt,
            "attn_b": bp,
            "z_cols": z_cols,
            "enc_cs": np.ascontiguousarray(encp[:, c * CS:(c + 1) * CS].astype(NP16)),
            "comb_wt": np.ascontiguousarray(np.concatenate(
                [comb_WT[c * CS:(c + 1) * CS], comb_WT[H + c * CS:H + (c + 1) * CS]],
                axis=0).astype(NP16)),
            "combx": np.ascontiguousarray(x[c * CS:(c + 1) * CS].reshape(2, 128).T.astype(NP16)),
            "comb_b": comb_bt,
            "wih_t": np.ascontiguousarray(w_ih[sel].T[perm].astype(NP16)),
            "whh_t": np.ascontiguousarray(w_hh[sel].T[perm].astype(NP16)),
            "h_cols": h_cols,
            "hsl": np.ascontiguousarray(h[c * CS:(c + 1) * CS].reshape(1, CS)),
            "bih": np.ascontiguousarray(b_ih[sel].reshape(1, GS)),
            "bhh": np.ascontiguousarray(b_hh[sel].reshape(1, GS)),
        })
    return in_maps


def kernel(**inputs):
    if "nc" not in _CACHE:
        _CACHE["nc"] = _build()
    nc = _CACHE["nc"]
    in_maps = _prep(inputs)
    res = bass_utils.run_bass_kernel_spmd(
        nc, in_maps, core_ids=list(range(NCORES)), **_CACHE.get("run_kwargs", {}))
    _CACHE["last_result"] = res

    h_full = np.concatenate(
        [np.asarray(res.results[c]["h_part"]).reshape(CS) for c in range(NCORES)])
    aw_full = np.asarray(res.results[0]["aw_full"]).reshape(LP)[:L]
    out = h_full.reshape(1, 1, H).astype(np.float32)
    return (out, out.copy(), aw_full.reshape(1, L).astype(np.float32))
